# revision 12
# baseline (speedup 1.0000x reference)
"""Trainium2 Bass kernel for nn_Decoder (2-layer LSTM decoder + vocab projection).

Computation (matches reference.py):
  embeds = emb[sentence]                      [B, T, E]
  x = concat(features, embeds[:, :-1])        [B, T, E]
  h0 = LSTM0(x), h1 = LSTM1(h0)               [B, T, H]
  out = (h1 @ fc_W.T + fc_b).transpose(0,2,1) [B, V, T]

Sharding: LSTM is replicated on all 8 cores (it is sequential in T and
streaming-bound, so batch-splitting does not reduce wall time); the fc
vocab dimension is sharded 8 ways (4000 rows per core, padded to 4096).
Each core writes its [B, 4096, T] logits slice; the host concatenates.

Device layout ("k-space"): every tensor that enters a matmul lives with
the contraction dim on partitions:  X[p, kc, ...] == X_full[kc*128+p, ...].
Gate chunks land g-on-partitions, so LSTM state (c, h) is k-aligned and
feeds the next step's matmul without any transpose.
"""

import numpy as np
import ml_dtypes

# ---------------------------------------------------------------------------
# Workaround: this walrus build caps instructions at ONE embedded sync wait
# ("Too many sync wait commands" in setupSyncWait); Tile routinely attaches
# several.  Post-process the serialized BIR: hoist excess waits of every
# instruction onto same-engine NoOp carriers inserted immediately before it.
# Semantics are identical (all waits still complete before the instruction
# executes on its engine).
# ---------------------------------------------------------------------------
import orjson
import concourse.tile as tile

_MAXW = 1


def _split_waits_json(b: bytes) -> bytes:
    d = orjson.loads(b)
    for f in d["functions"]:
        for blk in f["blocks"]:
            out = []
            for inst in blk["instructions"]:
                si = inst.get("sync_info")
                if si:
                    w = si.get("on_wait") or []
                    if len(w) > _MAXW:
                        for i, wt in enumerate(w[:-_MAXW]):
                            out.append(
                                {
                                    "debug": inst.get("debug", 0),
                                    "engine": inst["engine"],
                                    "ins": [],
                                    "outs": [],
                                    "name": f"{inst['name']}-hw{i}",
                                    "opcode": "NoOp",
                                    "sync_info": {"on_update": [], "on_wait": [wt]},
                                }
                            )
                        si["on_wait"] = w[-_MAXW:]
                out.append(inst)
            blk["instructions"] = out
    return orjson.dumps(d)


def _patch_serialization(nc):
    orig = nc.to_json_bytes
    nc.to_json_bytes = lambda: _split_waits_json(orig())
    return nc

import concourse.bass as bass
import concourse.mybir as mybir
from concourse.bass import ts, ds
from concourse.bass_utils import run_bass_kernel_spmd

F32 = mybir.dt.float32
BF16 = mybir.dt.bfloat16
AF = mybir.ActivationFunctionType
BF16_NP = ml_dtypes.bfloat16

E, H, V, B, T = 512, 512, 32000, 64, 32
G = 4 * H                    # 2048 gate rows per layer
KC = 4                       # 512 = 4 k-chunks of 128
NCORES = 8
VPAD = 4096                  # per-core vocab slice, padded from 4000
NTOK = B * T                 # 2048


def _build_nc():
    nc = bass.Bass()

    xT_d = nc.dram_tensor("xT", [128, KC, NTOK], BF16, kind="ExternalInput")
    wih0_d = nc.dram_tensor("wih0T", [128, KC, G], BF16, kind="ExternalInput")
    whh0_d = nc.dram_tensor("whh0T", [128, KC, G], BF16, kind="ExternalInput")
    wih1_d = nc.dram_tensor("wih1T", [128, KC, G], BF16, kind="ExternalInput")
    whh1_d = nc.dram_tensor("whh1T", [128, KC, G], BF16, kind="ExternalInput")
    b0_d = nc.dram_tensor("b0", [128, 16], F32, kind="ExternalInput")
    b1_d = nc.dram_tensor("b1", [128, 16], F32, kind="ExternalInput")
    fcw_d = nc.dram_tensor("fcwT", [128, KC, VPAD], BF16, kind="ExternalInput")
    fcb_d = nc.dram_tensor("fcb", [128, VPAD // 128], F32, kind="ExternalInput")
    out_d = nc.dram_tensor("out", [B, VPAD, T], F32, kind="ExternalOutput")

    with tile.TileContext(nc) as tc:
        with (
            tc.tile_pool(name="consts", bufs=1) as consts,
            tc.tile_pool(name="state", bufs=1) as state,
            tc.tile_pool(name="stage", bufs=4) as stage,
            tc.tile_pool(name="ps_gates", bufs=2, space="PSUM") as ps_gates,
            tc.tile_pool(name="ps_big", bufs=4, space="PSUM") as ps_big,
        ):
            # ---- constants live for the whole kernel ----
            b0_sb = consts.tile([128, 16], F32, tag="b0")
            nc.gpsimd.dma_start(out=b0_sb, in_=b0_d[:])
            b1_sb = consts.tile([128, 16], F32, tag="b1")
            nc.gpsimd.dma_start(out=b1_sb, in_=b1_d[:])
            fcb_sb = consts.tile([128, VPAD // 128], F32, tag="fcb")
            nc.gpsimd.dma_start(out=fcb_sb, in_=fcb_d[:])
            fcw_sb = consts.tile([128, KC, VPAD], BF16, tag="fcw")
            nc.gpsimd.dma_start(out=fcw_sb, in_=fcw_d[:])

            # h history: layer0 token-major (t, b) for the xp1 matmuls;
            # layer1 batch-major (b, t) for the fc matmuls / output DMA.
            hist0 = state.tile([128, KC, T, B], BF16, tag="hist0")
            hist1 = state.tile([128, KC, B, T], BF16, tag="hist1")
            cT = state.tile([128, KC, B], F32, tag="cT")
            gates = state.tile([128, 16, B], F32, tag="gates")
            tmp1 = state.tile([128, KC, B], F32, tag="tmp1")
            tmp2 = state.tile([128, KC, B], F32, tag="tmp2")
            tanh_c = state.tile([128, KC, B], F32, tag="tanh_c")

            def xp_phase(xp_sb, w_sb, rhs_slice, bias_sb):
                """xp[g, tok] = W_ih @ input  (+bias), stored bf16 in SBUF."""
                for g in range(16):
                    for n in range(4):
                        ps = ps_big.tile([128, 512], F32, tag="ps512")
                        for kc in range(KC):
                            nc.tensor.matmul(
                                ps,
                                w_sb[:, kc, ts(g, 128)],
                                rhs_slice(kc, n),
                                start=(kc == 0),
                                stop=(kc == KC - 1),
                            )
                        nc.scalar.activation(
                            out=xp_sb[:, g, ts(n, 512)],
                            in_=ps,
                            func=AF.Identity,
                            bias=bias_sb[:, g : g + 1],
                            scale=1.0,
                        )

            def rec_phase(xp_sb, whh_sb, hist_rd, hist_wr):
                """32 sequential LSTM steps; all tiles k-space."""
                for t in range(T):
                    if t == 0:
                        nc.scalar.activation(
                            gates[:, 0:8, :], xp_sb[:, 0:8, ts(t, B)], func=AF.Sigmoid
                        )
                        nc.scalar.activation(
                            gates[:, 8:12, :], xp_sb[:, 8:12, ts(t, B)], func=AF.Tanh
                        )
                        nc.scalar.activation(
                            gates[:, 12:16, :], xp_sb[:, 12:16, ts(t, B)], func=AF.Sigmoid
                        )
                        # c = i * g  (h=c=0 initially)
                        nc.vector.tensor_mul(cT, gates[:, 0:4, :], gates[:, 8:12, :])
                    else:
                        ps0 = ps_gates.tile([128, 8, B], F32, tag="ps0")
                        ps1 = ps_gates.tile([128, 8, B], F32, tag="ps1")
                        for half, ps in ((0, ps0), (1, ps1)):
                            for j in range(8):
                                gc = half * 8 + j
                                for kc in range(KC):
                                    nc.tensor.matmul(
                                        ps[:, j, :],
                                        whh_sb[:, kc, ts(gc, 128)],
                                        hist_rd(kc, t - 1),
                                        start=(kc == 0),
                                        stop=(kc == KC - 1),
                                    )
                        nc.vector.tensor_add(
                            gates[:, 0:8, :], ps0, xp_sb[:, 0:8, ts(t, B)]
                        )
                        nc.vector.tensor_add(
                            gates[:, 8:16, :], ps1, xp_sb[:, 8:16, ts(t, B)]
                        )
                        nc.scalar.activation(
                            gates[:, 0:8, :], gates[:, 0:8, :], func=AF.Sigmoid
                        )
                        nc.scalar.activation(
                            gates[:, 8:12, :], gates[:, 8:12, :], func=AF.Tanh
                        )
                        nc.scalar.activation(
                            gates[:, 12:16, :], gates[:, 12:16, :], func=AF.Sigmoid
                        )
                        nc.vector.tensor_mul(tmp1, gates[:, 0:4, :], gates[:, 8:12, :])
                        nc.vector.tensor_mul(tmp2, gates[:, 4:8, :], cT)
                        nc.vector.tensor_add(cT, tmp1, tmp2)
                    nc.scalar.activation(tanh_c, cT, func=AF.Tanh)
                    nc.vector.tensor_mul(hist_wr(t), gates[:, 12:16, :], tanh_c)

            # ================= layer 0 =================
            with tc.tile_pool(name="xp0pool", bufs=1) as xp0pool:
                xp0_sb = xp0pool.tile([128, 16, NTOK], BF16, tag="xp0")
                whh0_sb = xp0pool.tile([128, KC, G], BF16, tag="whh0")
                nc.gpsimd.dma_start(out=whh0_sb, in_=whh0_d[:])
                with tc.tile_pool(name="inpool0", bufs=1) as inpool:
                    xT_sb = inpool.tile([128, KC, NTOK], BF16, tag="xT")
                    nc.gpsimd.dma_start(out=xT_sb, in_=xT_d[:])
                    wih0_sb = inpool.tile([128, KC, G], BF16, tag="wih0")
                    nc.gpsimd.dma_start(out=wih0_sb, in_=wih0_d[:])
                    xp_phase(
                        xp0_sb,
                        wih0_sb,
                        lambda kc, n: xT_sb[:, kc, ts(n, 512)],
                        b0_sb,
                    )
                rec_phase(
                    xp0_sb,
                    whh0_sb,
                    lambda kc, t: hist0[:, kc, t, :],
                    lambda t: hist0[:, :, t, :],
                )

            # ================= layer 1 =================
            with tc.tile_pool(name="xp1pool", bufs=1) as xp1pool:
                xp1_sb = xp1pool.tile([128, 16, NTOK], BF16, tag="xp1")
                whh1_sb = xp1pool.tile([128, KC, G], BF16, tag="whh1")
                nc.gpsimd.dma_start(out=whh1_sb, in_=whh1_d[:])
                with tc.tile_pool(name="inpool1", bufs=1) as inpool:
                    wih1_sb = inpool.tile([128, KC, G], BF16, tag="wih1")
                    nc.gpsimd.dma_start(out=wih1_sb, in_=wih1_d[:])
                    xp_phase(
                        xp1_sb,
                        wih1_sb,
                        lambda kc, n: hist0[:, kc, ts(n, 8), :],
                        b1_sb,
                    )
                rec_phase(
                    xp1_sb,
                    whh1_sb,
                    lambda kc, t: hist1[:, kc, :, t],
                    lambda t: hist1[:, :, :, t],
                )

            # ================= fc =================
            for v in range(VPAD // 128):
                for n in range(4):
                    ps = ps_big.tile([128, 16, T], F32, tag="ps512")
                    for kc in range(KC):
                        nc.tensor.matmul(
                            ps,
                            fcw_sb[:, kc, ts(v, 128)],
                            hist1[:, kc, ts(n, 16), :],
                            start=(kc == 0),
                            stop=(kc == KC - 1),
                        )
                    ot = stage.tile([128, 16, T], F32, tag="ot")
                    nc.scalar.activation(
                        out=ot, in_=ps, func=AF.Identity, bias=fcb_sb[:, v : v + 1], scale=1.0
                    )
                    nc.sync.dma_start(
                        out=out_d[ds(16 * n, 16), ts(v, 128), :].rearrange(
                            "b v t -> v b t"
                        ),
                        in_=ot,
                    )
    return _patch_serialization(nc)


def _to_k128(W, dtype):
    """W [out_dim, K] -> [128, K//128, out_dim] with result[p,kc,g]=W[g,kc*128+p]."""
    K = W.shape[1]
    return np.ascontiguousarray(
        W.T.reshape(K // 128, 128, -1).transpose(1, 0, 2)
    ).astype(dtype)


_NC_CACHE = None
RUN_KWARGS = {}
LAST_RESULT = None


def kernel(
    sentence,
    features,
    lengths,
    emb,
    W_ih0,
    W_hh0,
    b_ih0,
    b_hh0,
    W_ih1,
    W_hh1,
    b_ih1,
    b_hh1,
    fc_W,
    fc_b,
):
    global _NC_CACHE
    sentence = np.asarray(sentence).astype(np.int64)
    features = np.asarray(features, dtype=np.float32)
    emb = np.asarray(emb, dtype=np.float32)

    # embedding gather + teacher forcing shift (host; pure data movement)
    embeds = emb[sentence[:, : T - 1]]                      # [B, T-1, E]
    x = np.concatenate([features[:, None, :], embeds], axis=1)  # [B, T, E]
    # token-major [k, tok] with tok = t*B + b
    xT = np.ascontiguousarray(x.transpose(2, 1, 0).reshape(E, NTOK))
    xT_p = np.ascontiguousarray(
        xT.reshape(KC, 128, NTOK).transpose(1, 0, 2)
    ).astype(BF16_NP)

    wih0 = _to_k128(np.asarray(W_ih0, np.float32), BF16_NP)
    whh0 = _to_k128(np.asarray(W_hh0, np.float32), BF16_NP)
    wih1 = _to_k128(np.asarray(W_ih1, np.float32), BF16_NP)
    whh1 = _to_k128(np.asarray(W_hh1, np.float32), BF16_NP)
    b0 = np.ascontiguousarray(
        (np.asarray(b_ih0, np.float32) + np.asarray(b_hh0, np.float32))
        .reshape(16, 128)
        .T
    )
    b1 = np.ascontiguousarray(
        (np.asarray(b_ih1, np.float32) + np.asarray(b_hh1, np.float32))
        .reshape(16, 128)
        .T
    )

    fc_W = np.asarray(fc_W, np.float32)
    fc_b = np.asarray(fc_b, np.float32)
    vloc = V // NCORES  # 4000 real rows per core, padded to VPAD

    common = {
        "xT": xT_p,
        "wih0T": wih0,
        "whh0T": whh0,
        "wih1T": wih1,
        "whh1T": whh1,
        "b0": b0,
        "b1": b1,
    }
    in_maps = []
    for c in range(NCORES):
        wslice = np.zeros((VPAD, E), np.float32)
        wslice[:vloc] = fc_W[c * vloc : (c + 1) * vloc]
        bslice = np.zeros(VPAD, np.float32)
        bslice[:vloc] = fc_b[c * vloc : (c + 1) * vloc]
        wc = _to_k128(wslice, BF16_NP)
        bc = np.ascontiguousarray(bslice.reshape(VPAD // 128, 128).T)
        in_maps.append({**common, "fcwT": wc, "fcb": bc})

    if _NC_CACHE is None:
        _NC_CACHE = _build_nc()

    global LAST_RESULT
    res = run_bass_kernel_spmd(
        _NC_CACHE, in_maps, core_ids=list(range(NCORES)), **RUN_KWARGS
    )
    LAST_RESULT = res
    full = np.concatenate(
        [res.results[c]["out"][:, :vloc, :] for c in range(NCORES)], axis=1
    )
    return np.ascontiguousarray(full)


# revision 15
# speedup vs baseline: 1.4960x; 1.4960x over previous
"""Trainium2 Bass kernel for nn_Decoder (2-layer LSTM decoder + vocab projection).

Computation (matches reference.py):
  embeds = emb[sentence]                      [B, T, E]
  x = concat(features, embeds[:, :-1])        [B, T, E]
  h0 = LSTM0(x), h1 = LSTM1(h0)               [B, T, H]
  out = (h1 @ fc_W.T + fc_b).transpose(0,2,1) [B, V, T]

Sharding: the LSTM is replicated on all 8 cores (it is sequential in T and
streaming/weight-load bound, so batch-splitting would not reduce wall time);
the fc vocab dimension is sharded 8 ways (4000 rows per core, padded to
4096).  Each core writes its [B, 4096, T] logits slice; the host
concatenates and trims.

Device layout ("k-space"): every tensor that enters a matmul lives with the
contraction dim on partitions:  X[p, kc, ...] == X_full[kc*128+p, ...].
Gate chunks land g-on-partitions, so the LSTM state (c, h) is k-aligned and
feeds the next step's matmul without any transpose.

Schedule: the two LSTM layers and the layer-1 input projection are software-
pipelined — emission order per step t is
  rec0(t) | xp1 chunk (every 4 steps) | rec1(t-5)
so each layer's post-matmul elementwise chain hides under the other
streams' matmuls.  xp tensors are staged in DRAM (bf16) to fit SBUF, and
the per-step xp slab is folded into the gates PSUM with an identity-weight
matmul, so ScalarE applies the nonlinearity directly from PSUM.
"""

import numpy as np
import ml_dtypes

# ---------------------------------------------------------------------------
# Workaround: this walrus build caps instructions at ONE embedded sync wait
# ("Too many sync wait commands" in setupSyncWait); Tile routinely attaches
# several.  Post-process the serialized BIR: hoist excess waits of every
# instruction onto same-engine NoOp carriers inserted immediately before it.
# Semantics are identical (all waits still complete before the instruction
# executes on its engine).
# ---------------------------------------------------------------------------
import orjson
import concourse.tile as tile

_MAXW = 1


def _split_waits_json(b: bytes) -> bytes:
    d = orjson.loads(b)
    for f in d["functions"]:
        for blk in f["blocks"]:
            out = []
            for inst in blk["instructions"]:
                si = inst.get("sync_info")
                if si:
                    w = si.get("on_wait") or []
                    if len(w) > _MAXW:
                        for i, wt in enumerate(w[:-_MAXW]):
                            out.append(
                                {
                                    "debug": inst.get("debug", 0),
                                    "engine": inst["engine"],
                                    "ins": [],
                                    "outs": [],
                                    "name": f"{inst['name']}-hw{i}",
                                    "opcode": "NoOp",
                                    "sync_info": {"on_update": [], "on_wait": [wt]},
                                }
                            )
                        si["on_wait"] = w[-_MAXW:]
                out.append(inst)
            blk["instructions"] = out
    return orjson.dumps(d)


def _patch_serialization(nc):
    orig = nc.to_json_bytes
    nc.to_json_bytes = lambda: _split_waits_json(orig())
    return nc


import concourse.bass as bass
import concourse.mybir as mybir
from concourse.bass import ts, ds
from concourse.bass_utils import run_bass_kernel_spmd

F32 = mybir.dt.float32
BF16 = mybir.dt.bfloat16
AF = mybir.ActivationFunctionType
BF16_NP = ml_dtypes.bfloat16

E, H, V, B, T = 512, 512, 32000, 64, 32
G = 4 * H                    # 2048 gate rows per layer
KC = 4                       # 512 = 4 k-chunks of 128
NCORES = 8
VPAD = 4096                  # per-core vocab slice, padded from 4000
NTOK = B * T                 # 2048
LAG = 5                      # rec1 runs LAG steps behind rec0


def _build_nc():
    nc = bass.Bass()

    xT_d = nc.dram_tensor("xT", [128, KC, NTOK], BF16, kind="ExternalInput")
    wih0_d = nc.dram_tensor("wih0T", [128, KC, G], BF16, kind="ExternalInput")
    whh0_d = nc.dram_tensor("whh0T", [128, KC, G], BF16, kind="ExternalInput")
    wih1_d = nc.dram_tensor("wih1T", [128, KC, G], BF16, kind="ExternalInput")
    whh1_d = nc.dram_tensor("whh1T", [128, KC, G], BF16, kind="ExternalInput")
    b0_d = nc.dram_tensor("b0", [128, 16], F32, kind="ExternalInput")
    b1_d = nc.dram_tensor("b1", [128, 16], F32, kind="ExternalInput")
    ident_d = nc.dram_tensor("ident", [128, 128], BF16, kind="ExternalInput")
    fcw_d = nc.dram_tensor("fcwT", [128, KC, VPAD], BF16, kind="ExternalInput")
    fcb_d = nc.dram_tensor("fcb", [128, VPAD // 128], F32, kind="ExternalInput")
    out_d = nc.dram_tensor("out", [B, VPAD, T], F32, kind="ExternalOutput")
    # DRAM staging for the (bias-folded) input projections, bf16
    xp0_d = nc.dram_tensor("xp0_stage", [128, 16, NTOK], BF16)
    xp1_d = nc.dram_tensor("xp1_stage", [128, 16, NTOK], BF16)

    with tile.TileContext(nc) as tc:
        with (
            tc.tile_pool(name="consts", bufs=1) as consts,
            tc.tile_pool(name="state", bufs=1) as state,
            tc.tile_pool(name="slab", bufs=4) as slab_pool,
            tc.tile_pool(name="xstage", bufs=4) as xstage,
            tc.tile_pool(name="fcstage", bufs=6) as fcstage,
            tc.tile_pool(name="ps_gates", bufs=2, space="PSUM") as ps_gates,
            tc.tile_pool(name="ps_big", bufs=4, space="PSUM") as ps_big,
        ):
            # ---- small constants ----
            b0_sb = consts.tile([128, 16], F32, tag="b0")
            nc.scalar.dma_start(out=b0_sb, in_=b0_d[:])
            b1_sb = consts.tile([128, 16], F32, tag="b1")
            nc.scalar.dma_start(out=b1_sb, in_=b1_d[:])
            fcb_sb = consts.tile([128, VPAD // 128], F32, tag="fcb")
            nc.scalar.dma_start(out=fcb_sb, in_=fcb_d[:])
            ident = consts.tile([128, 128], BF16, tag="ident")
            nc.scalar.dma_start(out=ident, in_=ident_d[:])

            # ---- histories ----
            hist0 = consts.tile([128, KC, T, B], BF16, tag="hist0")   # t-major
            hist1t = consts.tile([128, KC, T, B], BF16, tag="hist1t")  # t-major (rec)
            hist1b = consts.tile([128, KC, B, T], BF16, tag="hist1b")  # b-major (fc)

            # ---- per-layer state ----
            st = []
            for l in range(2):
                cT = state.tile([128, KC, B], F32, tag=f"cT{l}", name=f"cT{l}")
                gates = state.tile([128, 16, B], F32, tag=f"gates{l}", name=f"gates{l}")
                tmp1 = state.tile([128, KC, B], F32, tag=f"tmp1{l}", name=f"tmp1{l}")
                tmp2 = state.tile([128, KC, B], F32, tag=f"tmp2{l}", name=f"tmp2{l}")
                tanh_c = state.tile([128, KC, B], F32, tag=f"tanhc{l}", name=f"tanhc{l}")
                st.append(dict(cT=cT, gates=gates, tmp1=tmp1, tmp2=tmp2, tanh_c=tanh_c))

            def xp_chunk(w_sb, rhs_slice, bias_sb, xp_dram, n0, ntoks):
                """Compute xp[:, :, n0:n0+ntoks] = W.T@in (+bias) -> DRAM bf16."""
                for g in range(16):
                    ps = ps_big.tile([128, ntoks], F32, tag="ps512")
                    for kc in range(KC):
                        nc.tensor.matmul(
                            ps,
                            w_sb[:, kc, ts(g, 128)],
                            rhs_slice(kc, n0, ntoks),
                            start=(kc == 0),
                            stop=(kc == KC - 1),
                        )
                    xs = xstage.tile([128, ntoks], BF16, tag="xs")
                    nc.scalar.activation(
                        out=xs, in_=ps, func=AF.Identity,
                        bias=bias_sb[:, g : g + 1], scale=1.0,
                    )
                    nc.gpsimd.dma_start(out=xp_dram[:, g, ds(n0, ntoks)], in_=xs)

            def rec_step(l, t, whh_sb, xp_dram, hist_rd, hist_wr):
                s = st[l]
                # prefetch this step's xp slab (bias already folded in)
                xsl = slab_pool.tile([128, 16, B], BF16, tag=f"xsl{l}")
                nc.sync.dma_start(out=xsl, in_=xp_dram[:, :, ts(t, B)])
                ps0 = ps_gates.tile([128, 8, B], F32, tag="ps0")
                ps1 = ps_gates.tile([128, 8, B], F32, tag="ps1")
                for half, ps in ((0, ps0), (1, ps1)):
                    if t > 0:
                        for j in range(8):
                            gc = half * 8 + j
                            for kc in range(KC):
                                nc.tensor.matmul(
                                    ps[:, j, :],
                                    whh_sb[:, kc, ts(gc, 128)],
                                    hist_rd(kc, t - 1),
                                    start=(j == 0 and kc == 0),
                                    stop=False,
                                    skip_group_check=True,
                                )
                    # fold xp into the PSUM group via identity weights
                    nc.tensor.matmul(
                        ps,
                        ident,
                        xsl[:, ts(half, 8), :],
                        start=(t == 0),
                        stop=True,
                        skip_group_check=True,
                    )
                g = s["gates"]
                nc.scalar.activation(g[:, 0:8, :], ps0, func=AF.Sigmoid)
                nc.scalar.activation(g[:, 8:12, :], ps1[:, 0:4, :], func=AF.Tanh)
                nc.scalar.activation(g[:, 12:16, :], ps1[:, 4:8, :], func=AF.Sigmoid)
                if t == 0:
                    nc.vector.tensor_mul(s["cT"], g[:, 0:4, :], g[:, 8:12, :])
                else:
                    nc.vector.tensor_mul(s["tmp1"], g[:, 0:4, :], g[:, 8:12, :])
                    nc.vector.tensor_mul(s["tmp2"], g[:, 4:8, :], s["cT"])
                    nc.vector.tensor_add(s["cT"], s["tmp1"], s["tmp2"])
                nc.scalar.activation(s["tanh_c"], s["cT"], func=AF.Tanh)
                for wr in hist_wr(t):
                    nc.vector.tensor_mul(wr, g[:, 12:16, :], s["tanh_c"])

            with tc.tile_pool(name="wpool", bufs=1) as wpool:
                whh0_sb = wpool.tile([128, KC, G], BF16, tag="whh0")
                nc.gpsimd.dma_start(out=whh0_sb, in_=whh0_d[:])
                wih1_sb = wpool.tile([128, KC, G], BF16, tag="wih1")
                nc.gpsimd.dma_start(out=wih1_sb, in_=wih1_d[:])
                whh1_sb = wpool.tile([128, KC, G], BF16, tag="whh1")
                nc.gpsimd.dma_start(out=whh1_sb, in_=whh1_d[:])

                rec0 = dict(
                    whh_sb=whh0_sb,
                    xp_dram=xp0_d,
                    hist_rd=lambda kc, t: hist0[:, kc, t, :],
                    hist_wr=lambda t: [hist0[:, :, t, :]],
                )
                rec1 = dict(
                    whh_sb=whh1_sb,
                    xp_dram=xp1_d,
                    hist_rd=lambda kc, t: hist1t[:, kc, t, :],
                    hist_wr=lambda t: [hist1t[:, :, t, :], hist1b[:, :, :, t]],
                )

                with tc.tile_pool(name="inpool", bufs=1) as inpool:
                    xT_sb = inpool.tile([128, KC, NTOK], BF16, tag="xT")
                    nc.sync.dma_start(out=xT_sb, in_=xT_d[:])
                    wih0_sb = inpool.tile([128, KC, G], BF16, tag="wih0")
                    nc.sync.dma_start(out=wih0_sb, in_=wih0_d[:])

                    xp0_rhs = lambda kc, n0, nt: xT_sb[:, kc, ds(n0, nt)]
                    xp1_rhs = lambda kc, n0, nt: hist0[:, kc, ds(n0 // B, nt // B), :]

                    # xp0 chunk 0, then the software-pipelined steady state
                    xp_chunk(wih0_sb, xp0_rhs, b0_sb, xp0_d, 0, 512)
                    for t in range(T):
                        if t % 8 == 0 and t // 8 + 1 <= 3:
                            xp_chunk(
                                wih0_sb, xp0_rhs, b0_sb, xp0_d, (t // 8 + 1) * 512, 512
                            )
                        rec_step(0, t, **rec0)
                        if t % 4 == 3:
                            xp_chunk(
                                wih1_sb, xp1_rhs, b1_sb, xp1_d, (t // 4) * 256, 256
                            )
                        if t >= LAG:
                            rec_step(1, t - LAG, **rec1)
                for s_ in range(T - LAG, T):
                    rec_step(1, s_, **rec1)

            # ================= fc =================
            with tc.tile_pool(name="fcpool", bufs=1) as fcpool:
                fcw_sb = fcpool.tile([128, KC, VPAD], BF16, tag="fcw")
                nc.gpsimd.dma_start(out=fcw_sb, in_=fcw_d[:])
                dma_engines = [nc.sync, nc.scalar, nc.gpsimd]
                for v in range(VPAD // 128):
                    for n in range(4):
                        ps = ps_big.tile([128, 16, T], F32, tag="ps512")
                        for kc in range(KC):
                            nc.tensor.matmul(
                                ps,
                                fcw_sb[:, kc, ts(v, 128)],
                                hist1b[:, kc, ts(n, 16), :],
                                start=(kc == 0),
                                stop=(kc == KC - 1),
                            )
                        ot = fcstage.tile([128, 16, T], F32, tag="ot")
                        nc.scalar.activation(
                            out=ot, in_=ps, func=AF.Identity,
                            bias=fcb_sb[:, v : v + 1], scale=1.0,
                        )
                        eng = dma_engines[(v * 4 + n) % len(dma_engines)]
                        eng.dma_start(
                            out=out_d[ds(16 * n, 16), ts(v, 128), :].rearrange(
                                "b v t -> v b t"
                            ),
                            in_=ot,
                        )
    return _patch_serialization(nc)


def _to_k128(W, dtype):
    """W [out_dim, K] -> [128, K//128, out_dim] with result[p,kc,g]=W[g,kc*128+p]."""
    K = W.shape[1]
    return np.ascontiguousarray(
        W.T.reshape(K // 128, 128, -1).transpose(1, 0, 2)
    ).astype(dtype)


_NC_CACHE = None
RUN_KWARGS = {}
LAST_RESULT = None


def kernel(
    sentence,
    features,
    lengths,
    emb,
    W_ih0,
    W_hh0,
    b_ih0,
    b_hh0,
    W_ih1,
    W_hh1,
    b_ih1,
    b_hh1,
    fc_W,
    fc_b,
):
    global _NC_CACHE, LAST_RESULT
    sentence = np.asarray(sentence).astype(np.int64)
    features = np.asarray(features, dtype=np.float32)
    emb = np.asarray(emb, dtype=np.float32)

    # embedding gather + teacher forcing shift (host; pure data movement)
    embeds = emb[sentence[:, : T - 1]]                      # [B, T-1, E]
    x = np.concatenate([features[:, None, :], embeds], axis=1)  # [B, T, E]
    # token-major [k, tok] with tok = t*B + b
    xT = np.ascontiguousarray(x.transpose(2, 1, 0).reshape(E, NTOK))
    xT_p = np.ascontiguousarray(
        xT.reshape(KC, 128, NTOK).transpose(1, 0, 2)
    ).astype(BF16_NP)

    wih0 = _to_k128(np.asarray(W_ih0, np.float32), BF16_NP)
    whh0 = _to_k128(np.asarray(W_hh0, np.float32), BF16_NP)
    wih1 = _to_k128(np.asarray(W_ih1, np.float32), BF16_NP)
    whh1 = _to_k128(np.asarray(W_hh1, np.float32), BF16_NP)
    b0 = np.ascontiguousarray(
        (np.asarray(b_ih0, np.float32) + np.asarray(b_hh0, np.float32))
        .reshape(16, 128)
        .T
    )
    b1 = np.ascontiguousarray(
        (np.asarray(b_ih1, np.float32) + np.asarray(b_hh1, np.float32))
        .reshape(16, 128)
        .T
    )
    identity = np.eye(128, dtype=BF16_NP)

    fc_W = np.asarray(fc_W, np.float32)
    fc_b = np.asarray(fc_b, np.float32)
    vloc = V // NCORES  # 4000 real rows per core, padded to VPAD

    common = {
        "xT": xT_p,
        "wih0T": wih0,
        "whh0T": whh0,
        "wih1T": wih1,
        "whh1T": whh1,
        "b0": b0,
        "b1": b1,
        "ident": identity,
    }
    in_maps = []
    for c in range(NCORES):
        wslice = np.zeros((VPAD, E), np.float32)
        wslice[:vloc] = fc_W[c * vloc : (c + 1) * vloc]
        bslice = np.zeros(VPAD, np.float32)
        bslice[:vloc] = fc_b[c * vloc : (c + 1) * vloc]
        wc = _to_k128(wslice, BF16_NP)
        bc = np.ascontiguousarray(bslice.reshape(VPAD // 128, 128).T)
        in_maps.append({**common, "fcwT": wc, "fcb": bc})

    if _NC_CACHE is None:
        _NC_CACHE = _build_nc()

    res = run_bass_kernel_spmd(
        _NC_CACHE, in_maps, core_ids=list(range(NCORES)), **RUN_KWARGS
    )
    LAST_RESULT = res
    full = np.concatenate(
        [res.results[c]["out"][:, :vloc, :] for c in range(NCORES)], axis=1
    )
    return np.ascontiguousarray(full)


# revision 16
# speedup vs baseline: 1.6007x; 1.0700x over previous
"""Trainium2 Bass kernel for nn_Decoder (2-layer LSTM decoder + vocab projection).

Computation (matches reference.py):
  embeds = emb[sentence]                      [B, T, E]
  x = concat(features, embeds[:, :-1])        [B, T, E]
  h0 = LSTM0(x), h1 = LSTM1(h0)               [B, T, H]
  out = (h1 @ fc_W.T + fc_b).transpose(0,2,1) [B, V, T]

Sharding: the LSTM is replicated on all 8 cores (it is sequential in T and
streaming/weight-load bound, so batch-splitting would not reduce wall time);
the fc vocab dimension is sharded 8 ways (4000 rows per core, padded to
4096).  Each core writes its [B, 4096, T] logits slice; the host
concatenates and trims.

Device layout ("k-space"): every tensor that enters a matmul lives with the
contraction dim on partitions:  X[p, kc, ...] == X_full[kc*128+p, ...].
Gate chunks land g-on-partitions, so the LSTM state (c, h) is k-aligned and
feeds the next step's matmul without any transpose.

Schedule: the two LSTM layers and the layer-1 input projection are software-
pipelined — emission order per step t is
  rec0(t) | xp1 chunk (every 4 steps) | rec1(t-5)
so each layer's post-matmul elementwise chain hides under the other
streams' matmuls.  xp tensors are staged in DRAM (bf16) to fit SBUF, and
the per-step xp slab is folded into the gates PSUM with an identity-weight
matmul, so ScalarE applies the nonlinearity directly from PSUM.
"""

import numpy as np
import ml_dtypes

# ---------------------------------------------------------------------------
# Workaround: this walrus build caps instructions at ONE embedded sync wait
# ("Too many sync wait commands" in setupSyncWait); Tile routinely attaches
# several.  Post-process the serialized BIR: hoist excess waits of every
# instruction onto same-engine NoOp carriers inserted immediately before it.
# Semantics are identical (all waits still complete before the instruction
# executes on its engine).
# ---------------------------------------------------------------------------
import orjson
import concourse.tile as tile

_MAXW = 1


def _split_waits_json(b: bytes) -> bytes:
    d = orjson.loads(b)
    for f in d["functions"]:
        for blk in f["blocks"]:
            out = []
            for inst in blk["instructions"]:
                si = inst.get("sync_info")
                if si:
                    w = si.get("on_wait") or []
                    if len(w) > _MAXW:
                        for i, wt in enumerate(w[:-_MAXW]):
                            out.append(
                                {
                                    "debug": inst.get("debug", 0),
                                    "engine": inst["engine"],
                                    "ins": [],
                                    "outs": [],
                                    "name": f"{inst['name']}-hw{i}",
                                    "opcode": "NoOp",
                                    "sync_info": {"on_update": [], "on_wait": [wt]},
                                }
                            )
                        si["on_wait"] = w[-_MAXW:]
                out.append(inst)
            blk["instructions"] = out
    return orjson.dumps(d)


def _patch_serialization(nc):
    orig = nc.to_json_bytes
    nc.to_json_bytes = lambda: _split_waits_json(orig())
    return nc


import concourse.bass as bass
import concourse.mybir as mybir
from concourse.bass import ts, ds
from concourse.bass_utils import run_bass_kernel_spmd

F32 = mybir.dt.float32
BF16 = mybir.dt.bfloat16
AF = mybir.ActivationFunctionType
BF16_NP = ml_dtypes.bfloat16

E, H, V, B, T = 512, 512, 32000, 64, 32
G = 4 * H                    # 2048 gate rows per layer
KC = 4                       # 512 = 4 k-chunks of 128
NCORES = 8
VPAD = 4096                  # per-core vocab slice, padded from 4000
NTOK = B * T                 # 2048
LAG = 5                      # rec1 runs LAG steps behind rec0


def _build_nc():
    nc = bass.Bass()

    xT_d = nc.dram_tensor("xT", [128, KC, NTOK], BF16, kind="ExternalInput")
    wih0_d = nc.dram_tensor("wih0T", [128, KC, G], BF16, kind="ExternalInput")
    whh0_d = nc.dram_tensor("whh0T", [128, KC, G], BF16, kind="ExternalInput")
    wih1_d = nc.dram_tensor("wih1T", [128, KC, G], BF16, kind="ExternalInput")
    whh1_d = nc.dram_tensor("whh1T", [128, KC, G], BF16, kind="ExternalInput")
    b0_d = nc.dram_tensor("b0", [128, 16], F32, kind="ExternalInput")
    b1_d = nc.dram_tensor("b1", [128, 16], F32, kind="ExternalInput")
    ident_d = nc.dram_tensor("ident", [128, 128], BF16, kind="ExternalInput")
    fcw_d = nc.dram_tensor("fcwT", [128, KC, VPAD], BF16, kind="ExternalInput")
    fcb_d = nc.dram_tensor("fcb", [128, VPAD // 128], F32, kind="ExternalInput")
    out_d = nc.dram_tensor("out", [B, VPAD, T], F32, kind="ExternalOutput")

    with tile.TileContext(nc) as tc:
        with (
            tc.tile_pool(name="consts", bufs=1) as consts,
            tc.tile_pool(name="state", bufs=1) as state,
            tc.tile_pool(name="fcstage", bufs=6) as fcstage,
            tc.tile_pool(name="ps_gates", bufs=2, space="PSUM") as ps_gates,
            tc.tile_pool(name="ps_big", bufs=4, space="PSUM") as ps_big,
        ):
            # ---- small constants ----
            b0_sb = consts.tile([128, 16], F32, tag="b0")
            nc.scalar.dma_start(out=b0_sb, in_=b0_d[:])
            b1_sb = consts.tile([128, 16], F32, tag="b1")
            nc.scalar.dma_start(out=b1_sb, in_=b1_d[:])
            fcb_sb = consts.tile([128, VPAD // 128], F32, tag="fcb")
            nc.scalar.dma_start(out=fcb_sb, in_=fcb_d[:])
            ident = consts.tile([128, 128], BF16, tag="ident")
            nc.scalar.dma_start(out=ident, in_=ident_d[:])

            # ---- histories ----
            hist0 = consts.tile([128, KC, T, B], BF16, tag="hist0")   # t-major
            hist1t = consts.tile([128, KC, T, B], BF16, tag="hist1t")  # t-major (rec)
            hist1b = consts.tile([128, KC, B, T], BF16, tag="hist1b")  # b-major (fc)
            # SBUF rings for the bias-folded input projections (8 slabs each)
            xp0r = consts.tile([128, 8, 16, B], BF16, tag="xp0r")
            xp1r = consts.tile([128, 8, 16, B], BF16, tag="xp1r")

            # ---- per-layer state ----
            st = []
            for l in range(2):
                cT = state.tile([128, KC, B], F32, tag=f"cT{l}", name=f"cT{l}")
                gates = state.tile([128, 16, B], F32, tag=f"gates{l}", name=f"gates{l}")
                tmp1 = state.tile([128, KC, B], F32, tag=f"tmp1{l}", name=f"tmp1{l}")
                tmp2 = state.tile([128, KC, B], F32, tag=f"tmp2{l}", name=f"tmp2{l}")
                tanh_c = state.tile([128, KC, B], F32, tag=f"tanhc{l}", name=f"tanhc{l}")
                st.append(dict(cT=cT, gates=gates, tmp1=tmp1, tmp2=tmp2, tanh_c=tanh_c))

            def xp_chunk(w_sb, rhs_slice, bias_sb, ring, c):
                """xp chunk c = slabs 4c..4c+3 -> ring slots (4c)%8.. via ScalarE."""
                n0, ntoks = c * 4 * B, 4 * B
                s0 = (4 * c) % 8
                for g in range(16):
                    ps = ps_big.tile([128, 4, B], F32, tag="ps512")
                    for kc in range(KC):
                        nc.tensor.matmul(
                            ps,
                            w_sb[:, kc, ts(g, 128)],
                            rhs_slice(kc, n0, ntoks),
                            start=(kc == 0),
                            stop=(kc == KC - 1),
                        )
                    nc.scalar.activation(
                        out=ring[:, ds(s0, 4), g, :], in_=ps, func=AF.Identity,
                        bias=bias_sb[:, g : g + 1], scale=1.0,
                    )

            def rec_step(l, t, whh_sb, ring, hist_rd, hist_wr):
                s = st[l]
                xsl = ring[:, t % 8, :, :]   # [128, 16, B], bias already folded
                ps0 = ps_gates.tile([128, 8, B], F32, tag="ps0")
                ps1 = ps_gates.tile([128, 8, B], F32, tag="ps1")
                for half, ps in ((0, ps0), (1, ps1)):
                    if t > 0:
                        for j in range(8):
                            gc = half * 8 + j
                            for kc in range(KC):
                                nc.tensor.matmul(
                                    ps[:, j, :],
                                    whh_sb[:, kc, ts(gc, 128)],
                                    hist_rd(kc, t - 1),
                                    start=(j == 0 and kc == 0),
                                    stop=False,
                                    skip_group_check=True,
                                )
                    # fold xp into the PSUM group via identity weights
                    nc.tensor.matmul(
                        ps,
                        ident,
                        xsl[:, ts(half, 8), :],
                        start=(t == 0),
                        stop=True,
                        skip_group_check=True,
                    )
                g = s["gates"]
                nc.scalar.activation(g[:, 0:8, :], ps0, func=AF.Sigmoid)
                nc.scalar.activation(g[:, 8:12, :], ps1[:, 0:4, :], func=AF.Tanh)
                nc.scalar.activation(g[:, 12:16, :], ps1[:, 4:8, :], func=AF.Sigmoid)
                if t == 0:
                    nc.vector.tensor_mul(s["cT"], g[:, 0:4, :], g[:, 8:12, :])
                else:
                    nc.vector.tensor_mul(s["tmp1"], g[:, 0:4, :], g[:, 8:12, :])
                    nc.vector.tensor_mul(s["tmp2"], g[:, 4:8, :], s["cT"])
                    nc.vector.tensor_add(s["cT"], s["tmp1"], s["tmp2"])
                nc.scalar.activation(s["tanh_c"], s["cT"], func=AF.Tanh)
                for wr in hist_wr(t):
                    nc.vector.tensor_mul(wr, g[:, 12:16, :], s["tanh_c"])

            with tc.tile_pool(name="wpool", bufs=1) as wpool:
                whh0_sb = wpool.tile([128, KC, G], BF16, tag="whh0")
                nc.gpsimd.dma_start(out=whh0_sb, in_=whh0_d[:])
                wih1_sb = wpool.tile([128, KC, G], BF16, tag="wih1")
                nc.gpsimd.dma_start(out=wih1_sb, in_=wih1_d[:])
                whh1_sb = wpool.tile([128, KC, G], BF16, tag="whh1")
                nc.gpsimd.dma_start(out=whh1_sb, in_=whh1_d[:])

                rec0 = dict(
                    whh_sb=whh0_sb,
                    ring=xp0r,
                    hist_rd=lambda kc, t: hist0[:, kc, t, :],
                    hist_wr=lambda t: [hist0[:, :, t, :]],
                )
                rec1 = dict(
                    whh_sb=whh1_sb,
                    ring=xp1r,
                    hist_rd=lambda kc, t: hist1t[:, kc, t, :],
                    hist_wr=lambda t: [hist1t[:, :, t, :], hist1b[:, :, :, t]],
                )

                with tc.tile_pool(name="inpool", bufs=1) as inpool:
                    xT_sb = inpool.tile([128, KC, NTOK], BF16, tag="xT")
                    wih0_sb = inpool.tile([128, KC, G], BF16, tag="wih0")
                    for kc in range(KC):
                        nc.sync.dma_start(out=xT_sb[:, kc, :], in_=xT_d[:, kc, :])
                        nc.sync.dma_start(out=wih0_sb[:, kc, :], in_=wih0_d[:, kc, :])

                    xp0_rhs = lambda kc, n0, nt: xT_sb[:, kc, ds(n0, nt)]
                    xp1_rhs = lambda kc, n0, nt: hist0[:, kc, ds(n0 // B, nt // B), :]

                    # xp0 chunk 0, then the software-pipelined steady state
                    xp_chunk(wih0_sb, xp0_rhs, b0_sb, xp0r, 0)
                    for t in range(T):
                        if t % 4 == 0 and t // 4 + 1 <= 7:
                            xp_chunk(wih0_sb, xp0_rhs, b0_sb, xp0r, t // 4 + 1)
                        rec_step(0, t, **rec0)
                        if t % 4 == 3:
                            xp_chunk(wih1_sb, xp1_rhs, b1_sb, xp1r, t // 4)
                        if t >= LAG:
                            rec_step(1, t - LAG, **rec1)
                for s_ in range(T - LAG, T):
                    rec_step(1, s_, **rec1)

            # ================= fc =================
            with tc.tile_pool(name="fcpool", bufs=1) as fcpool:
                fcw_sb = fcpool.tile([128, KC, VPAD], BF16, tag="fcw")
                nc.gpsimd.dma_start(out=fcw_sb, in_=fcw_d[:])
                dma_engines = [nc.sync, nc.scalar]
                for v in range(VPAD // 128):
                    for n in range(4):
                        ps = ps_big.tile([128, 16, T], F32, tag="ps512")
                        for kc in range(KC):
                            nc.tensor.matmul(
                                ps,
                                fcw_sb[:, kc, ts(v, 128)],
                                hist1b[:, kc, ts(n, 16), :],
                                start=(kc == 0),
                                stop=(kc == KC - 1),
                            )
                        ot = fcstage.tile([128, 16, T], F32, tag="ot")
                        nc.scalar.activation(
                            out=ot, in_=ps, func=AF.Identity,
                            bias=fcb_sb[:, v : v + 1], scale=1.0,
                        )
                        eng = dma_engines[(v * 4 + n) % len(dma_engines)]
                        eng.dma_start(
                            out=out_d[ds(16 * n, 16), ts(v, 128), :].rearrange(
                                "b v t -> v b t"
                            ),
                            in_=ot,
                        )
    return _patch_serialization(nc)


def _to_k128(W, dtype):
    """W [out_dim, K] -> [128, K//128, out_dim] with result[p,kc,g]=W[g,kc*128+p]."""
    K = W.shape[1]
    return np.ascontiguousarray(
        W.T.reshape(K // 128, 128, -1).transpose(1, 0, 2)
    ).astype(dtype)


_NC_CACHE = None
RUN_KWARGS = {}
LAST_RESULT = None


def kernel(
    sentence,
    features,
    lengths,
    emb,
    W_ih0,
    W_hh0,
    b_ih0,
    b_hh0,
    W_ih1,
    W_hh1,
    b_ih1,
    b_hh1,
    fc_W,
    fc_b,
):
    global _NC_CACHE, LAST_RESULT
    sentence = np.asarray(sentence).astype(np.int64)
    features = np.asarray(features, dtype=np.float32)
    emb = np.asarray(emb, dtype=np.float32)

    # embedding gather + teacher forcing shift (host; pure data movement)
    embeds = emb[sentence[:, : T - 1]]                      # [B, T-1, E]
    x = np.concatenate([features[:, None, :], embeds], axis=1)  # [B, T, E]
    # token-major [k, tok] with tok = t*B + b
    xT = np.ascontiguousarray(x.transpose(2, 1, 0).reshape(E, NTOK))
    xT_p = np.ascontiguousarray(
        xT.reshape(KC, 128, NTOK).transpose(1, 0, 2)
    ).astype(BF16_NP)

    wih0 = _to_k128(np.asarray(W_ih0, np.float32), BF16_NP)
    whh0 = _to_k128(np.asarray(W_hh0, np.float32), BF16_NP)
    wih1 = _to_k128(np.asarray(W_ih1, np.float32), BF16_NP)
    whh1 = _to_k128(np.asarray(W_hh1, np.float32), BF16_NP)
    b0 = np.ascontiguousarray(
        (np.asarray(b_ih0, np.float32) + np.asarray(b_hh0, np.float32))
        .reshape(16, 128)
        .T
    )
    b1 = np.ascontiguousarray(
        (np.asarray(b_ih1, np.float32) + np.asarray(b_hh1, np.float32))
        .reshape(16, 128)
        .T
    )
    identity = np.eye(128, dtype=BF16_NP)

    fc_W = np.asarray(fc_W, np.float32)
    fc_b = np.asarray(fc_b, np.float32)
    vloc = V // NCORES  # 4000 real rows per core, padded to VPAD

    common = {
        "xT": xT_p,
        "wih0T": wih0,
        "whh0T": whh0,
        "wih1T": wih1,
        "whh1T": whh1,
        "b0": b0,
        "b1": b1,
        "ident": identity,
    }
    in_maps = []
    for c in range(NCORES):
        wslice = np.zeros((VPAD, E), np.float32)
        wslice[:vloc] = fc_W[c * vloc : (c + 1) * vloc]
        bslice = np.zeros(VPAD, np.float32)
        bslice[:vloc] = fc_b[c * vloc : (c + 1) * vloc]
        wc = _to_k128(wslice, BF16_NP)
        bc = np.ascontiguousarray(bslice.reshape(VPAD // 128, 128).T)
        in_maps.append({**common, "fcwT": wc, "fcb": bc})

    if _NC_CACHE is None:
        _NC_CACHE = _build_nc()

    res = run_bass_kernel_spmd(
        _NC_CACHE, in_maps, core_ids=list(range(NCORES)), **RUN_KWARGS
    )
    LAST_RESULT = res
    full = np.concatenate(
        [res.results[c]["out"][:, :vloc, :] for c in range(NCORES)], axis=1
    )
    return np.ascontiguousarray(full)
